# revision 9
# baseline (speedup 1.0000x reference)
"""Trainium2 Bass kernel for nn_NodeTaskHead (8-core SPMD).

Sharding: core c = 2*b + g handles batch b and heads [16g, 16g+16).
No collectives: each core returns per-head unnormalized force rows and
softmax partition sums; the host divides, sums heads/groups, adds bf.

Math (per batch b, head h, axis c):
  force[n,c] = sum_h sum_m probs_h[n,m] * delta[n,m,c] * omega_h[m,c] + bf[c]
  omega_h[m,c] = sum_d v_h[m,d] * Wf[c, h*24+d]
  probs_h[n,m] = P_h[m,n] / Z_h[n],  P = exp(qk^T scores^T) * exp(bias^T),
  Z_h[n] = sum_m P_h[m,n]
All m-contractions are TensorE M=1 column reductions; the (m,n) transposed
layout keeps m on partitions so no on-chip transposes are needed.
"""

import sys

sys.path.insert(0, "/opt/trn_rl_repo")

import numpy as np
import ml_dtypes

import concourse.bacc as bacc
import concourse.tile as tile
from concourse import mybir
from concourse.bass_utils import run_bass_kernel_spmd

B, N, E, H, D = 4, 512, 768, 32, 24
HPC = 16  # heads per core
DP = 32  # padded per-head dim
EP = HPC * DP  # 512 padded qkv width per core
KP = 896  # padded contraction: 768 + 1 bias row + 127 zeros = 7*128
KCH = KP // 128
NT = N // 128  # 4 tiles along token/key axis

f32 = mybir.dt.float32
f32r = mybir.dt.float32r
bf16 = mybir.dt.bfloat16
BF = ml_dtypes.bfloat16

_CACHE = {}


def _build_module(variant="full"):
    nc = bacc.Bacc()
    qT_d = nc.declare_dram_parameter("qT", [KP, N], f32r, isOutput=False)
    wqT_d = nc.declare_dram_parameter("wqT", [KP, EP], f32r, isOutput=False)
    wkT_d = nc.declare_dram_parameter("wkT", [KP, EP], f32r, isOutput=False)
    wvT_d = nc.declare_dram_parameter("wvT", [KP, EP], f32r, isOutput=False)
    wfT_d = nc.declare_dram_parameter("wfT", [EP, 16], bf16, isOutput=False)
    eb_d = nc.declare_dram_parameter("eb", [HPC, N, N], bf16, isOutput=False)
    dl_d = nc.declare_dram_parameter("dl", [3, N, N], bf16, isOutput=False)
    out_d = nc.declare_dram_parameter("out", [HPC, 4, N], f32, isOutput=True)

    with tile.TileContext(nc) as tc:
        with (
            tc.tile_pool(name="wpool", bufs=1) as wpool,
            tc.tile_pool(name="qkv", bufs=1) as qkvpool,
            tc.tile_pool(name="stat", bufs=1) as statpool,
            tc.tile_pool(name="bias", bufs=8) as biaspool,
            tc.tile_pool(name="work", bufs=4) as workpool,
            tc.tile_pool(name="dwork", bufs=6) as dworkpool,
            tc.tile_pool(name="fout", bufs=2) as foutpool,
            tc.tile_pool(name="pbig", bufs=4, space="PSUM") as pbig,
            tc.tile_pool(name="pom", bufs=2, space="PSUM") as pom,
        ):
            # ---- static loads -------------------------------------------------
            tqT = [wpool.tile([128, N], f32r, name=f"tqT{_k}") for _k in range(KCH)]
            for k in range(KCH):
                nc.sync.dma_start(tqT[k][:], qT_d[128 * k : 128 * (k + 1), :])
            tw = {}
            for nm, dparam in (("q", wqT_d), ("k", wkT_d), ("v", wvT_d)):
                tw[nm] = [wpool.tile([128, EP], f32r, name=f"tw_{nm}{_k}") for _k in range(KCH)]
                for k in range(KCH):
                    nc.sync.dma_start(
                        tw[nm][k][:], dparam[128 * k : 128 * (k + 1), :]
                    )
            twf = [
                statpool.tile([128, 16], bf16, name=f"twf{_t}")
                for _t in range(NT)
            ]
            for t in range(NT):
                nc.sync.dma_start(
                    twf[t][:], wfT_d[128 * t : 128 * (t + 1), :]
                )
            tdl = [
                [
                    statpool.tile(
                        [128, N], bf16, name=f"tdl{_c}_{_mt}"
                    )
                    for _mt in range(NT)
                ]
                for _c in range(3)
            ]
            for c in range(3):
                for mt in range(NT):
                    nc.sync.dma_start(
                        tdl[c][mt][:], dl_d[c, 128 * mt : 128 * (mt + 1), :]
                    )
            ones32 = statpool.tile([128, 32], bf16)
            nc.vector.memset(ones32[:], 0.0)
            nc.vector.memset(ones32[:, 0:1], 1.0)

            # ---- qkv projections: out[hd_pad, n] tiles -----------------------
            # qkvT[x][t] [128, N] f32r, x in q/k/v, t tiles heads 4t..4t+3
            qkvT = {
                nm: [
                    qkvpool.tile(
                        [128, N],
                        bf16 if nm == "v" else f32r,
                        name=f"{nm}T{_t}",
                    )
                    for _t in range(NT)
                ]
                for nm in ("q", "k", "v")
            }
            for nm in ("q", "k", "v"):
                for t in range(NT):
                    ps = pbig.tile([128, N], f32, tag="big")
                    for k in range(KCH):
                        nc.tensor.matmul(
                            ps[:],
                            tw[nm][k][:, 128 * t : 128 * (t + 1)],
                            tqT[k][:],
                            start=(k == 0),
                            stop=(k == KCH - 1),
                        )
                    nc.scalar.copy(qkvT[nm][t][:], ps[:])

            # ---- omega: per m-chunk, all 16 heads x 3 axes -------------------
            # omegas[mt] [128, 48] bf16; col 12*t + 3*j + c for head 4t+j
            omegas = [
                statpool.tile([128, 48 * 32], bf16, name=f"om{_mt}")
                for _mt in range(NT)
            ]
            for mt in range(NT):
                nc.vector.memset(omegas[mt][:], 0.0)
            do_omega = variant in ("full", "nofc")
            do_fc = variant == "full"
            for t in range(NT if do_omega else 0):
                for mt in range(NT):
                    pw = pom.tile([128, 16], f32, tag="om")
                    nc.tensor.matmul(
                        pw[:],
                        qkvT["v"][t][:, 128 * mt : 128 * (mt + 1)],
                        twf[t][:],
                        start=True,
                        stop=True,
                    )
                    om_view = omegas[mt].rearrange("p (i s) -> p i s", s=32)
                    for j in range(4):
                        nc.scalar.copy(
                            om_view[:, 12 * t + 3 * j : 12 * t + 3 * j + 3, 0:1],
                            pw[:, 4 * j : 4 * j + 3].rearrange(
                                "p (c s) -> p c s", s=1
                            ),
                        )

            # ---- per-head attention + force reductions -----------------------
            for h in range(HPC):
                t, j = divmod(h, 4)
                rq = qkvT["q"][t][32 * j : 32 * (j + 1), :]
                F = pbig.tile([128, N], f32, tag="big")
                for mt in range(NT):
                    # scoresT[m_chunk, n] = k_h[m]^T . q_h[n]
                    ps = pbig.tile([128, N], f32, tag="big")
                    nc.tensor.matmul(
                        ps[:],
                        qkvT["k"][t][
                            32 * j : 32 * (j + 1), 128 * mt : 128 * (mt + 1)
                        ],
                        rq,
                        start=True,
                        stop=True,
                        tile_position=(32 * j, 0),
                    )
                    texp = workpool.tile([128, N], bf16, tag="exp")
                    nc.scalar.activation(
                        texp[:], ps[:], mybir.ActivationFunctionType.Exp
                    )
                    teb = biaspool.tile([128, N], bf16, tag="eb")
                    nc.sync.dma_start(
                        teb[:], eb_d[h, 128 * mt : 128 * (mt + 1), :]
                    )
                    tP = workpool.tile([128, N], bf16, tag="P")
                    nc.vector.tensor_tensor(
                        out=tP[:], in0=texp[:], in1=teb[:], op=mybir.AluOpType.mult
                    )
                    # Z row
                    nc.tensor.matmul(
                        F[0:32, :],
                        ones32[:],
                        tP[:],
                        start=(mt == 0),
                        stop=(mt == NT - 1),
                        tile_position=(0, 0),
                        skip_group_check=True,
                    )
                    for c in range(3 if do_fc else 0):
                        tD = dworkpool.tile([128, N], bf16, tag="D")
                        nc.vector.tensor_tensor(
                            out=tD[:],
                            in0=tP[:],
                            in1=tdl[c][mt][:],
                            op=mybir.AluOpType.mult,
                        )
                        col = 12 * t + 3 * j + c
                        nc.tensor.matmul(
                            F[32 * (c + 1) : 32 * (c + 2), :],
                            omegas[mt][:, 32 * col : 32 * (col + 1)],
                            tD[:],
                            start=(mt == 0),
                            stop=(mt == NT - 1),
                            tile_position=(0, 32 * (c + 1)),
                            skip_group_check=True,
                        )
                sF = foutpool.tile([128, N], f32, tag="sF")
                nc.vector.tensor_copy(sF[:], F[:])
                nc.sync.dma_start(out_d[h], sF[0:128:32, :])

    nc.finalize()
    return nc


def _prep_core_inputs(core, query, attn_bias, delta_pos, Wq, bq, Wk, bk, Wv, bv, Wf):
    b, g = divmod(core, 2)
    hs = slice(g * HPC, (g + 1) * HPC)
    scal = np.float32(D ** -0.5)

    def wt_pad(W, bias):
        Wh = W.reshape(H, D, E)[hs]  # (16, 24, 768)
        tmp = np.zeros((HPC, DP, E), np.float32)
        tmp[:, :D] = Wh
        out = np.zeros((KP, EP), np.float32)
        out[:E] = tmp.reshape(EP, E).T
        bt = np.zeros((HPC, DP), np.float32)
        bt[:, :D] = bias.reshape(H, D)[hs]
        out[E] = bt.reshape(EP)
        return out

    qp = np.zeros((KP, N), np.float32)
    qp[:E] = query[b].T
    qp[E] = 1.0

    # block-diagonal per vT tile: row t*128 + 32j + dd, col 4j + c holds
    # Wf[c, (g*16 + 4t + j)*24 + dd]
    wf = np.zeros((NT, 4, DP, 4, 4), np.float32)
    wfh = Wf.reshape(3, H, D)[:, hs]  # (3, 16, 24)
    for t in range(NT):
        for j in range(4):
            for c in range(3):
                wf[t, j, :D, j, c] = wfh[c, 4 * t + j]
    wf = wf.reshape(EP, 16)

    eb = np.exp(
        attn_bias.reshape(B, H, N, N)[b, hs].transpose(0, 2, 1)
    ).astype(BF)
    dl = np.ascontiguousarray(delta_pos[b].transpose(2, 1, 0)).astype(BF)

    return {
        "qT": np.ascontiguousarray(qp),
        "wqT": np.ascontiguousarray(wt_pad(Wq * scal, bq * scal)),
        "wkT": np.ascontiguousarray(wt_pad(Wk, bk)),
        "wvT": np.ascontiguousarray(wt_pad(Wv, bv)),
        "wfT": np.ascontiguousarray(wf).astype(BF),
        "eb": eb,
        "dl": dl,
    }


def kernel(query, attn_bias, delta_pos, Wq, bq, Wk, bk, Wv, bv, Wf, bf):
    query = np.asarray(query, np.float32)
    attn_bias = np.asarray(attn_bias, np.float32)
    delta_pos = np.asarray(delta_pos, np.float32)

    if "nc" not in _CACHE:
        _CACHE["nc"] = _build_module()
    nc = _CACHE["nc"]

    in_maps = [
        _prep_core_inputs(
            c, query, attn_bias, delta_pos, Wq, bq, Wk, bk, Wv, bv, Wf
        )
        for c in range(8)
    ]
    res = run_bass_kernel_spmd(nc, in_maps, list(range(8)))

    force = np.zeros((B, N, 3), np.float32)
    for core in range(8):
        b = core // 2
        o = res.results[core]["out"].astype(np.float64)  # (16, 4, 512)
        Z = o[:, 0, :]  # (16, n)
        Fc = o[:, 1:4, :]  # (16, 3, n)
        force[b] += (Fc / Z[:, None, :]).sum(0).T.astype(np.float32)
    force += np.asarray(bf, np.float32)[None, None, :]
    return force
